# revision 1
# baseline (speedup 1.0000x reference)
"""Trainium2 Bass kernel for DBFLinear:
    y = ((x * s0) @ unpack(bp1).T * s2) @ unpack(bp3).T * s4 + bias

Strategy: data-parallel over batch across 8 cores (weights replicated, no
collectives). Per core: unpack the bit-packed +/-1 weights on device
(DVE bitwise_and + ACT Sign), transpose weight blocks with the DMA xbar,
run both GEMMs weight-stationary (fp16, fp32 PSUM accumulation). scaling0
is folded into the unpacked W1 (+/-s0 is exact in fp16), scaling2 into the
h eviction, scaling4+bias into the y eviction — all per-partition ACT ops.
The device emits y.T per batch shard; the host transposes while unsharding.
"""

import sys

import numpy as np

sys.path.insert(0, "/opt/trn_rl_repo")

import concourse.bass as bass
import concourse.mybir as mybir
import concourse.tile as tile
from concourse.tile import add_dep_helper
from concourse import bacc
from concourse.bass_utils import run_bass_kernel_spmd

N_CORES = 8
B_FULL, IN, MID, OUT = 8192, 4096, 4096, 4096
P = 128
FD = 512  # matmul moving-operand free dim (1 PSUM bank of fp32)
QCH = 1024  # unpack quarter width (weight elements per DVE/ACT op)
N_WARM = 800  # HAM warm-up matmuls


def build_program(b=B_FULL // N_CORES, in_=IN, mid=MID, out=OUT):
    """Build the per-core Bass program. Returns the Bass object."""
    in_k, mid_k, out_k = in_ // P, mid // P, out // P
    nbc = 2  # batch processed as two halves
    fd = b // nbc
    assert fd <= FD, (b, fd)
    uch = min(QCH, in_, mid)

    nc = bacc.Bacc(num_devices=N_CORES)
    x_d = nc.dram_tensor("x", [b, in_], mybir.dt.float16, kind="ExternalInput")
    bp1_d = nc.dram_tensor("bp1", [mid, in_ // 8], mybir.dt.int32, kind="ExternalInput")
    bp3_d = nc.dram_tensor("bp3", [out, mid // 8], mybir.dt.int32, kind="ExternalInput")
    mask_d = nc.dram_tensor("mask", [P, 8], mybir.dt.int32, kind="ExternalInput")
    s0r_d = nc.dram_tensor("s0rep", [P, in_], mybir.dt.float16, kind="ExternalInput")
    s2_d = nc.dram_tensor("s2", [P, mid_k], mybir.dt.float32, kind="ExternalInput")
    s4_d = nc.dram_tensor("s4", [P, out_k], mybir.dt.float32, kind="ExternalInput")
    bias_d = nc.dram_tensor("bias", [P, out_k], mybir.dt.float32, kind="ExternalInput")
    yT_d = nc.dram_tensor("yT", [out, b], mybir.dt.float16, kind="ExternalOutput")

    Act = mybir.ActivationFunctionType

    with tile.TileContext(nc) as tc:
        with (
            tc.tile_pool(name="big", bufs=1) as big,
            tc.tile_pool(name="consts", bufs=1) as consts,
            tc.tile_pool(name="wpipe", bufs=2) as wpipe,
            tc.tile_pool(name="psum", bufs=4, space="PSUM") as psum,
        ):
            mask_t = consts.tile([P, 8], mybir.dt.int32)
            s0r_t = consts.tile([P, in_], mybir.dt.float16)
            s2_t = consts.tile([P, mid_k], mybir.dt.float32)
            s4_t = consts.tile([P, out_k], mybir.dt.float32)
            bias_t = consts.tile([P, out_k], mybir.dt.float32)
            neg_half = consts.tile([P, 1], mybir.dt.float32)
            for t, d in (
                (mask_t, mask_d),
                (s0r_t, s0r_d),
                (s2_t, s2_d),
                (s4_t, s4_d),
                (bias_t, bias_d),
            ):
                nc.gpsimd.dma_start(t[:], d[:])
            nc.vector.memset(neg_half[:], -0.5)

            # Warm the PE HAM clock gate with cheap junk matmuls while the
            # input pipeline fills, so the real stream starts at 2.4 GHz.
            junk = mask_t[:].bitcast(mybir.dt.float16)  # [P, 16] arbitrary bits
            warm_ps = psum.tile([P, 16], mybir.dt.float32, tag="warm")
            for _ in range(N_WARM):
                nc.tensor.matmul(warm_ps[:16, :], junk, junk, start=True, stop=True)

            _last_tr = [None]

            def load_bytes(bp_d, m, k_blocks, eng=None):
                kb = k_blocks * P // 8  # bytes per row
                byt = wpipe.tile([P, kb], mybir.dt.int32, tag="bytes", bufs=4)
                (eng or nc.sync).dma_start(byt[:], bp_d[m * P : (m + 1) * P, :])
                return byt

            def unpack_quarters(byt, k_blocks, scale_s0):
                """Unpack a loaded 128-row byte block into its transposed
                [P, k_blocks, P] weight tile, quarter by quarter.
                scale_s0: also multiply by the replicated scaling0 row."""
                wT = wpipe.tile([P, k_blocks, P], mybir.dt.float16, tag="wT", bufs=4)
                for c0 in range(0, k_blocks * P, uch):
                    nb = uch // 8
                    b0 = c0 // 8
                    masked = wpipe.tile([P, uch], mybir.dt.int32, tag="masked", bufs=3)
                    in0 = byt[:, b0 : b0 + nb][:, :, None].broadcast_to([P, nb, 8])
                    in1 = mask_t[:][:, None, :].broadcast_to([P, nb, 8])
                    nc.vector.tensor_tensor(
                        masked[:].rearrange("p (b j) -> p b j", j=8),
                        in0,
                        in1,
                        mybir.AluOpType.bitwise_and,
                    )
                    wq = wpipe.tile([P, uch], mybir.dt.float16, tag="wnat", bufs=4)
                    nc.scalar.activation(
                        wq[:], masked[:], Act.Sign, bias=neg_half[:, 0:1]
                    )
                    if scale_s0:
                        nc.gpsimd.tensor_tensor(
                            wq[:], wq[:], s0r_t[:, c0 : c0 + uch],
                            mybir.AluOpType.mult,
                        )
                    _last_tr[0] = nc.sync.dma_start_transpose(
                        wT[:, c0 // P : (c0 + uch) // P, :], wq[:]
                    ).ins
                return wT

            def unpack_wT(bp_d, m, k_blocks, scale_s0):
                return unpack_quarters(load_bytes(bp_d, m, k_blocks), k_blocks, scale_s0)

            # x.T in two batch halves: xH[h][p, k, r] = x[h*b/2 + r, 128k + p].
            # Band-split whole-half transposes read DRAM contiguously; no
            # scaling needed (scaling0 lives in W1).
            half = b // 2
            xH = [
                big.tile([P, in_k, half], mybir.dt.float16, tag=f"xT{h}", name=f"xh{h}")
                for h in range(2)
            ]

            def x_bands(h, after=None):
                # One full-width transpose per half: the DRAM read is fully
                # contiguous (whole rows), and 1024 xbar tiles keeps the DMA
                # semaphore threshold within the ISA field.
                tr = nc.sync.dma_start_transpose(
                    xH[h][:], x_d[h * half : (h + 1) * half, :]
                )
                if after is not None:
                    add_dep_helper(tr.ins, after, reason="x half-2 after startup wT")

            # Startup: prefetch byte blocks, transpose the first x half, then
            # unpack the first START_BLOCKS weight blocks, then the second x
            # half. The PE runs c0 passes of blocks 0..3 against the first x
            # half while the second is still transposing.
            SB = min(4, mid_k)
            x_bands(0)
            byts = [load_bytes(bp1_d, m, in_k, eng=nc.gpsimd) for m in range(SB)]
            wTs = [unpack_quarters(byts[m], in_k, True) for m in range(SB)]
            xh1_anchor = _last_tr[0]

            hT = big.tile([P, mid_k, b], mybir.dt.float16)

            def g1_pass(m, wT, c):
                ps = psum.tile([P, fd], mybir.dt.float32, tag="ps")
                for k in range(in_k):
                    nc.tensor.matmul(
                        ps[:],
                        wT[:, k, :],
                        xH[c][:, k, :],
                        start=(k == 0),
                        stop=(k == in_k - 1),
                    )
                nc.scalar.activation(
                    hT[:, m, c * fd : (c + 1) * fd],
                    ps[:],
                    Act.Copy,
                    scale=s2_t[:, m : m + 1],
                )

            # c-major startup over the first SB blocks; the second x half
            # transposes while the first-half passes run on the PE.
            for m in range(SB):
                g1_pass(m, wTs[m], 0)
            x_bands(1, after=xh1_anchor)
            for c in range(1, nbc):
                for m in range(SB):
                    g1_pass(m, wTs[m], c)

            # Unified steady loop: GEMM1 blocks SB.., then GEMM2 blocks, with
            # weight unpack prefetched two blocks ahead.
            n_blocks = mid_k + out_k

            def mk(jj):
                if jj >= n_blocks:
                    return None
                if jj < mid_k:
                    return unpack_wT(bp1_d, jj, in_k, True)
                return unpack_wT(bp3_d, jj - mid_k, mid_k, False)

            # GEMM2 output staging: groups of blocks buffered in the (dead)
            # x-half SBUF slots, stored with one DMA per group; the final
            # group is kept small so the tail store is short.
            yT_v = yT_d.rearrange("(g p) c -> p g c", p=P)
            ygroups = []
            _o = 0
            while _o < out_k:
                rem = out_k - _o
                if rem > 8:
                    n = 8
                elif rem > 2:
                    n = rem - 2
                else:
                    n = rem
                ygroups.append((_o, n))
                _o += n
            o2group = {}
            for gi_, (gs, gn) in enumerate(ygroups):
                for oo in range(gs, gs + gn):
                    o2group[oo] = (gi_, gs, gn)
            yt_g = None
            pend = [mk(SB), mk(SB + 1)]
            for j in range(SB, n_blocks):
                wT = pend.pop(0)
                pend.append(mk(j + 2))
                if j < mid_k:  # GEMM1 block
                    for c in range(nbc):
                        g1_pass(j, wT, c)
                else:  # GEMM2 block
                    o = j - mid_k
                    gi_, gstart, glen = o2group[o]
                    if o == gstart:
                        yt_g = big.tile(
                            [P, glen, b], mybir.dt.float16,
                            tag=f"xT{gi_ % 2}", name=f"ytg{o}",
                        )
                    for c in range(nbc):
                        ps = psum.tile([P, fd], mybir.dt.float32, tag="ps")
                        for k in range(mid_k):
                            nc.tensor.matmul(
                                ps[:],
                                wT[:, k, :],
                                hT[:, k, c * fd : (c + 1) * fd],
                                start=(k == 0),
                                stop=(k == mid_k - 1),
                            )
                        nc.scalar.activation(
                            yt_g[:, o - gstart, c * fd : (c + 1) * fd],
                            ps[:],
                            Act.Identity,
                            bias=bias_t[:, o : o + 1],
                            scale=s4_t[:, o : o + 1],
                        )
                    if o == gstart + glen - 1:
                        nc.sync.dma_start(
                            yT_v[:, gstart : gstart + glen, :], yt_g[:]
                        )

    nc.compile()
    return nc


def make_in_maps(x, scaling0, bp1, scaling2, bp3, scaling4, bias, n_cores=N_CORES):
    b_full, in_ = x.shape
    mid = scaling2.shape[0]
    out = scaling4.shape[0]
    b = b_full // n_cores

    mask = (1 << (7 - np.arange(8, dtype=np.int32)))[None, :].repeat(P, 0)
    mask = np.ascontiguousarray(mask.astype(np.int32))

    def pcol(v):
        return np.ascontiguousarray(v.astype(np.float32).reshape(-1, P).T)

    shared = {
        "bp1": np.ascontiguousarray(bp1.reshape(mid, in_ // 8)),
        "bp3": np.ascontiguousarray(bp3.reshape(out, mid // 8)),
        "mask": mask,
        "s0rep": np.ascontiguousarray(
            np.broadcast_to(scaling0.astype(np.float16)[None, :], (P, in_))
        ),
        "s2": pcol(scaling2),
        "s4": pcol(scaling4),
        "bias": pcol(bias),
    }
    return [
        {"x": np.ascontiguousarray(x[c * b : (c + 1) * b]), **shared}
        for c in range(n_cores)
    ]


_PROGRAM_CACHE = {}


def run(x, scaling0, bp1, scaling2, bp3, scaling4, bias, **spmd_kwargs):
    """Compile (cached) + run on 8 cores; returns (y, BassKernelResults)."""
    if "nc" not in _PROGRAM_CACHE:
        _PROGRAM_CACHE["nc"] = build_program()
    nc = _PROGRAM_CACHE["nc"]
    in_maps = make_in_maps(x, scaling0, bp1, scaling2, bp3, scaling4, bias)
    res = run_bass_kernel_spmd(nc, in_maps, core_ids=list(range(N_CORES)), **spmd_kwargs)
    b = x.shape[0] // N_CORES
    y = np.empty((x.shape[0], scaling4.shape[0]), dtype=np.float16)
    for c in range(N_CORES):
        y[c * b : (c + 1) * b] = res.results[c]["yT"].T
    return y, res


def kernel(x, scaling0, bp1, scaling2, bp3, scaling4, bias):
    y, _ = run(x, scaling0, bp1, scaling2, bp3, scaling4, bias)
    return y



# revision 2
# speedup vs baseline: 1.0238x; 1.0238x over previous
"""Trainium2 Bass kernel for DBFLinear using fp8 DoubleRow matmuls:
    y = ((x * s0) @ unpack(bp1).T * s2) @ unpack(bp3).T * s4 + bias

Strategy: data-parallel over batch across 8 cores. The +/-1 weights are
exact in fp8e4, so both GEMMs run in fp8 with perf_mode=DoubleRow (2 fp8
weights per PE cell, 256-deep contraction per instruction). Activations
are split hi/lo: z = e4m3(16*z) + e4m3(16*z - hi) on the host (scale 16
keeps the lo part clear of the PE's fp8 subnormal flush); h likewise on
device (ACT evict * s2 -> fp16, cast hi, subtract lo). The lo corrections
run as extra pairs in the same PSUM accumulation chain: L1 pairs for
GEMM1 (x side), L2 for GEMM2 (h side) - the L knobs trade rel-err vs PE
time.

Host prep: bp1/bp3 are bit-transposed (numpy) so W1T/W3T unpack directly
on device as [i(part), j(free)] with NO on-device transposes: packed
int32 words -> DVE bitwise_and against per-bit masks -> ACT Sign -> fp8
pair-major weight tiles [P, 2, 128] (the int32 sign bit j%32==24 is fixed
up with a Sign(scale=-1) pass). z is pre-scaled/quantized/transposed on
host and uploaded as int8 (bitcast to fp8 on device).
"""

import sys

import numpy as np
import ml_dtypes

sys.path.insert(0, "/opt/trn_rl_repo")

import concourse.bass as bass
import concourse.mybir as mybir
import concourse.tile as tile
from concourse import bacc
from concourse.bass_utils import run_bass_kernel_spmd

N_CORES = 8
B_FULL, IN, MID, OUT = 8192, 4096, 4096, 4096
P = 128
FD = 512               # moving cols per matmul (1 PSUM bank of fp32)
L1 = 8                 # x-side lo-correction pairs (contraction sorted by s0)
L2 = 8                 # h-side lo-correction pairs (contraction sorted by s2)
ZS = 16.0              # host scale on z before fp8 quantization
N_WARM = 800           # HAM warm-up matmuls

F8 = mybir.dt.float8e4
NPF8 = ml_dtypes.float8_e4m3
DR = mybir.MatmulPerfMode.DoubleRow


def dedup_ldweights(nc):
    """Drop redundant InstLdweights: consecutive PE matmuls sharing the same
    stationary AP only need the first load (a weight-buffer rewrite always
    puts a RAW semaphore wait on its first reader, so waitless repeats are
    provably redundant)."""
    for f in nc.m.functions:
        for blk in f.blocks:
            insts = list(blk.instructions)
            drop_idx = []
            last_sig = None
            for idx, inst in enumerate(insts):
                tn = type(inst).__name__
                if tn == "InstLdweights":
                    si = inst.sync_info
                    clean = si is None or (len(si.on_wait) == 0
                                           and len(si.on_update) == 0)
                    sig = (str(inst.ins[0]), str(inst.perf_mode))
                    if clean and sig == last_sig:
                        drop_idx.append(idx)
                    else:
                        last_sig = sig
                elif tn == "InstMatmult":
                    if inst.is_transpose:
                        last_sig = None
                else:
                    try:
                        eng = inst.engine
                    except Exception:
                        eng = None
                    if eng == mybir.EngineType.PE:
                        last_sig = None
            for idx in reversed(drop_idx):
                del blk.instructions[idx]


def build_program(b=B_FULL // N_CORES, in_=IN, mid=MID, out=OUT):
    in_k, mid_k, out_k = in_ // P, mid // P, out // P
    in_pr, mid_pr = in_k // 2, mid_k // 2
    nbc = b // FD

    nc = bacc.Bacc(num_devices=N_CORES)
    zhi_d = nc.dram_tensor("zhi", [in_, b], mybir.dt.int8, kind="ExternalInput")
    zlo_d = nc.dram_tensor("zlo", [256 * L1, b], mybir.dt.int8, kind="ExternalInput")
    w1b_d = nc.dram_tensor("w1b", [in_, mid // 32], mybir.dt.int32, kind="ExternalInput")
    w3b_d = nc.dram_tensor("w3b", [mid, out // 32], mybir.dt.int32, kind="ExternalInput")
    mask_d = nc.dram_tensor("mask", [P, 32], mybir.dt.int32, kind="ExternalInput")
    s2_d = nc.dram_tensor("s2", [P, mid_k], mybir.dt.float32, kind="ExternalInput")
    s4_d = nc.dram_tensor("s4", [P, out_k], mybir.dt.float32, kind="ExternalInput")
    bias_d = nc.dram_tensor("bias", [P, out_k], mybir.dt.float32, kind="ExternalInput")
    yT_d = nc.dram_tensor("yT", [out, b], mybir.dt.float16, kind="ExternalOutput")

    Act = mybir.ActivationFunctionType

    with tile.TileContext(nc) as tc:
        with (
            tc.tile_pool(name="big", bufs=1) as big,
            tc.tile_pool(name="consts", bufs=1) as consts,
            tc.tile_pool(name="wpipe", bufs=4) as wpipe,
            tc.tile_pool(name="evpipe", bufs=3) as evpipe,
            tc.tile_pool(name="psum", bufs=3, space="PSUM") as psum,
        ):
            mask_t = consts.tile([P, 32], mybir.dt.int32)
            s2_t = consts.tile([P, mid_k], mybir.dt.float32)
            s4_t = consts.tile([P, out_k], mybir.dt.float32)
            bias_t = consts.tile([P, out_k], mybir.dt.float32)
            neg_half = consts.tile([P, 1], mybir.dt.float32)
            for t, d in ((mask_t, mask_d), (s2_t, s2_d), (s4_t, s4_d),
                         (bias_t, bias_d)):
                nc.gpsimd.dma_start(t[:], d[:])
            nc.vector.memset(neg_half[:], -0.5)

            # Resident inputs: quantized z (hi/lo), packed weight bits.
            zhi = big.tile([P, in_k, b], mybir.dt.int8, name="zhi")
            zlo = big.tile([P, 2 * L1, b], mybir.dt.int8, name="zlo")
            by1 = big.tile([P, in_k, mid // 32], mybir.dt.int32, name="by1")
            by3 = big.tile([P, mid_k, out // 32], mybir.dt.int32, name="by3")
            zhi_r = zhi_d.rearrange("(k p) c -> p k c", p=P)
            zlo_r = zlo_d.rearrange("(k p) c -> p k c", p=P)
            by1_r = w1b_d.rearrange("(k p) w -> p k w", p=P)
            by3_r = w3b_d.rearrange("(k p) w -> p k w", p=P)
            for q0 in range(0, in_k, 8):
                s = slice(q0, q0 + 8)
                nc.gpsimd.dma_start(by1[:, s, :], by1_r[:, s, :])
                nc.sync.dma_start(zhi[:, s, :], zhi_r[:, s, :])
            for q0 in range(0, 2 * L1, 8):
                s = slice(q0, min(q0 + 8, 2 * L1))
                nc.sync.dma_start(zlo[:, s, :], zlo_r[:, s, :])
            for q0 in range(0, mid_k, 8):
                s = slice(q0, q0 + 8)
                nc.gpsimd.dma_start(by3[:, s, :], by3_r[:, s, :])
            zhi_v = zhi[:].bitcast(F8)
            zlo_v = zlo[:].bitcast(F8)

            # h (transposed, fp8 hi/lo) built by GEMM1, consumed by GEMM2.
            hhi = big.tile([P, mid_k, b], mybir.dt.int8, name="hhi")
            hlo = big.tile([P, 2 * L2, b], mybir.dt.int8, name="hlo")
            hhi_v = hhi[:].bitcast(F8)
            hlo_v = hlo[:].bitcast(F8)

            # Warm the PE clock gate while the input pipeline fills.
            junk = mask_t[:].bitcast(mybir.dt.float16)  # [P, 64] junk bits
            warm_ps = psum.tile([P, 16], mybir.dt.float32, tag="warm", bufs=1)
            for _ in range(N_WARM):
                nc.tensor.matmul(warm_ps[:16, :], junk[:, :16], junk[:, :16],
                                 start=True, stop=True)

            def unpack_block(byt, k_blocks, m):
                """Unpack weight block m: [P, k_blocks, 128] fp8 pair-major
                from packed int32 words byt[:, kb, m*4 : m*4+4]."""
                wT = wpipe.tile([P, k_blocks, P], F8, tag="wT")
                qk = 8  # k-blocks per quarter -> [P, 1024] ops
                for q0 in range(0, k_blocks, qk):
                    masked = wpipe.tile([P, qk * P], mybir.dt.int32, tag="masked",
                                        bufs=3)
                    in0 = byt[:, q0:q0 + qk, 4 * m:4 * m + 4][:, :, :, None]
                    in0 = in0.broadcast_to([P, qk, 4, 32])
                    in1 = mask_t[:][:, None, None, :].broadcast_to([P, qk, 4, 32])
                    nc.vector.tensor_tensor(
                        masked[:].rearrange("p (k w j) -> p k w j", w=4, j=32),
                        in0, in1, mybir.AluOpType.bitwise_and)
                    nc.scalar.activation(
                        wT[:, q0:q0 + qk, :].rearrange("p k j -> p (k j)"),
                        masked[:], Act.Sign, bias=neg_half[:, 0:1])
                # int32 sign bit: j % 32 == 24 columns, one pass per block
                nc.scalar.activation(
                    wT[:].rearrange("p k (w j) -> p k w j", j=32)[:, :, :, 24],
                    byt[:, 0:k_blocks, 4 * m:4 * m + 4],
                    Act.Sign, bias=neg_half[:, 0:1], scale=-1.0)
                return wT

            def gemm(n_blocks, pairs, lo_pairs, byt, mov_hi, mov_lo, evict):
                # lo-corrected pairs sit at even kp slots (host interleaves
                # high-energy columns there); compact lo tile index = kp//2.
                # kp-outer ordering: the 2 batch chunks and the hi/lo passes
                # for one k-pair run back-to-back on the same stationary, so
                # the compiler's ldw-opt drops the redundant LDWEIGHTS.
                pend = [unpack_block(byt, 2 * pairs, 0),
                        unpack_block(byt, 2 * pairs, 1)]
                for m in range(n_blocks):
                    wT = pend.pop(0)
                    if m + 2 < n_blocks:
                        pend.append(unpack_block(byt, 2 * pairs, m + 2))
                    pss = [psum.tile([P, FD], mybir.dt.float32, tag=f"ps{c}",
                                     name=f"psc{c}") for c in range(nbc)]
                    for kp in range(pairs):
                        wap = wT[:, 2 * kp:2 * kp + 2, :]
                        hi_last = (kp == pairs - 1)
                        for c in range(nbc):
                            nc.tensor.matmul(
                                pss[c][:], wap,
                                mov_hi[:, 2 * kp:2 * kp + 2,
                                       c * FD:(c + 1) * FD],
                                start=(kp == 0), stop=hi_last, perf_mode=DR)
                        if kp % 2 == 0 and kp // 2 < lo_pairs:
                            lo = kp // 2
                            for c in range(nbc):
                                nc.tensor.matmul(
                                    pss[c][:], wap,
                                    mov_lo[:, 2 * lo:2 * lo + 2,
                                           c * FD:(c + 1) * FD],
                                    start=False, stop=False, perf_mode=DR)
                    for c in range(nbc):
                        evict(m, c, slice(c * FD, (c + 1) * FD), pss[c])

            # ---- GEMM1: h = (z @ W1T) * s2, quantized hi/lo into hhi/hlo.
            def evict1(m, c, cs, ps):
                hs = evpipe.tile([P, FD], mybir.dt.float16, tag="hs")
                nc.scalar.activation(hs[:], ps[:], Act.Copy,
                                     scale=s2_t[:, m:m + 1])
                nc.gpsimd.tensor_copy(hhi_v[:, m, cs], hs[:])
                if (m // 2) % 2 == 0 and m // 4 < L2:
                    lo_m = (m // 4) * 2 + m % 2
                    nc.gpsimd.tensor_tensor(hlo_v[:, lo_m, cs], hs[:],
                                            hhi_v[:, m, cs],
                                            mybir.AluOpType.subtract)

            gemm(mid_k, in_pr, L1, by1, zhi_v, zlo_v, evict1)

            # ---- GEMM2: y = (h @ W3T) * s4 + bias -> yT staging -> DRAM.
            yT_v = yT_d.rearrange("(g p) c -> p g c", p=P)
            GRP = 4
            ygroups = []
            _g0 = 0
            while _g0 < out_k:
                rem = out_k - _g0
                n = GRP if rem > 4 else (2 if rem > 2 else 1)
                ygroups.append((_g0, n))
                _g0 += n
            o2group = {}
            for gs, gn in ygroups:
                for oo in range(gs, gs + gn):
                    o2group[oo] = (gs, gn)
            yt_g = [None]

            def evict2(m, c, cs, ps):
                gs, gn = o2group[m]
                if m == gs and c == 0:
                    yt_g[0] = big.tile([P, gn, b], mybir.dt.float16,
                                       tag=f"yg{(gs // GRP) % 2}",
                                       name=f"ytg{gs}")
                nc.scalar.activation(yt_g[0][:, m - gs, cs], ps[:], Act.Identity,
                                     bias=bias_t[:, m:m + 1],
                                     scale=s4_t[:, m:m + 1])
                if m == gs + gn - 1 and c == nbc - 1:
                    nc.sync.dma_start(yT_v[:, gs:gs + gn, :], yt_g[0][:])

            gemm(out_k, mid_pr, L2, by3, hhi_v, hlo_v, evict2)

    nc.compile()
    dedup_ldweights(nc)
    return nc


def host_prep(x, scaling0, bp1, scaling2, bp3, scaling4, bias,
              n_cores=N_CORES):
    b = x.shape[0] // n_cores

    def bit_transpose(bp, rows, cols, row_perm=None, col_perm=None):
        """bp: int32 array of packed bytes (one byte per int32, row-major
        [rows, cols/8], MSB-first). Returns [cols, rows/32] int32 words of
        the transposed (optionally permuted) bit-matrix (little-endian byte
        order, MSB-first in each byte)."""
        bts = bp.astype(np.uint8).reshape(rows, cols // 8)
        bits = np.unpackbits(bts, axis=1)            # [rows, cols]
        if row_perm is not None:
            bits = bits[row_perm]
        if col_perm is not None:
            bits = bits[:, col_perm]
        pk = np.packbits(np.ascontiguousarray(bits.T), axis=1)  # [cols, rows/8]
        return np.ascontiguousarray(pk).view(np.int32)  # [cols, rows/32]

    # Sort the contraction dims by scale, then interleave 256-wide pair
    # blocks (top-energy half at even pair slots, bottom at odd): the lo
    # corrections cover the even slots (err_tail ~ (1-f)^3 for uniform
    # scales) while the power profile stays flat at ~us scale.
    def interleave_pairs(perm_sorted, n_lo):
        g = perm_sorted.reshape(-1, 256)           # pair blocks, desc energy
        npr = g.shape[0]
        out = np.empty_like(g)
        out[0:2 * n_lo:2] = g[:n_lo]               # top half -> even slots
        out[1:2 * n_lo:2] = g[n_lo:2 * n_lo]       # bottom -> odd slots
        if 2 * n_lo < npr:
            out[2 * n_lo:] = g[2 * n_lo:]
        return out.reshape(-1)

    p1 = interleave_pairs(np.argsort(-scaling0.astype(np.float32)), L1)
    p2 = interleave_pairs(np.argsort(-scaling2.astype(np.float32)), L2)
    w1b = bit_transpose(bp1, MID, IN, row_perm=p2, col_perm=p1)
    w3b = bit_transpose(bp3, OUT, MID, col_perm=p2)

    # masks: bit(j) = 8*(j//8) + 7 - j%8 ; j == 24 -> int32 sign bit, fixup
    mask = np.zeros(32, np.int64)
    for j in range(32):
        bit = 8 * (j // 8) + 7 - (j % 8)
        mask[j] = 0 if bit == 31 else (1 << bit)
    mask = np.ascontiguousarray(
        mask.astype(np.int32)[None, :].repeat(P, 0))

    z = (x.astype(np.float32) * scaling0.astype(np.float32)[None, :])[:, p1] * ZS
    z_hi = z.astype(NPF8)
    z_lo = (z - z_hi.astype(np.float32)).astype(NPF8)
    zT_hi = np.ascontiguousarray(z_hi.T.view(np.int8))  # [IN, B]
    lo_rows = z_lo.T.view(np.int8).reshape(-1, 256, z_lo.shape[0])
    zT_lo = np.ascontiguousarray(
        lo_rows[0:2 * L1:2].reshape(-1, z_lo.shape[0]))

    def pcol(v, scale=1.0):
        return np.ascontiguousarray(
            (v.astype(np.float32) * scale).reshape(-1, P).T)

    shared = {
        "w1b": w1b, "w3b": w3b, "mask": mask,
        "s2": pcol(scaling2[p2], 1.0 / ZS),
        "s4": pcol(scaling4),
        "bias": pcol(bias),
    }
    return [
        {"zhi": np.ascontiguousarray(zT_hi[:, c * b:(c + 1) * b]),
         "zlo": np.ascontiguousarray(zT_lo[:, c * b:(c + 1) * b]),
         **shared}
        for c in range(n_cores)
    ]


_PROGRAM_CACHE = {}


def run(x, scaling0, bp1, scaling2, bp3, scaling4, bias, **spmd_kwargs):
    if "nc" not in _PROGRAM_CACHE:
        _PROGRAM_CACHE["nc"] = build_program()
    nc = _PROGRAM_CACHE["nc"]
    in_maps = host_prep(x, scaling0, bp1, scaling2, bp3, scaling4, bias)
    res = run_bass_kernel_spmd(nc, in_maps, core_ids=list(range(N_CORES)),
                               **spmd_kwargs)
    b = x.shape[0] // N_CORES
    y = np.empty((x.shape[0], scaling4.shape[0]), dtype=np.float16)
    for c in range(N_CORES):
        y[c * b:(c + 1) * b] = res.results[c]["yT"].T
    return y, res


def kernel(x, scaling0, bp1, scaling2, bp3, scaling4, bias):
    y, _ = run(x, scaling0, bp1, scaling2, bp3, scaling4, bias)
    return y
